# revision 12
# baseline (speedup 1.0000x reference)
"""DCT blur (nn_DCTBlur) on Trainium2, 8 NeuronCores, data-parallel over batch.

out[b,c] = (D @ x[b,c] @ D^T) * exp(-fsq * s[b]),  s[b] = 0.125 * 40**(2*t[b])

Per core: 8 batches x 3 channels = 24 images of 512x512.
Each image: two matmul stages on the PE array.
  Stage 1:  Y^T = X^T @ D^T  via matmul(out=Y^T, lhsT=X_blk, rhs=D^T_blk)
  Stage 2:  Z   = Y  @ D^T   via matmul(out=Z,  lhsT=Y^T_blk, rhs=D^T_blk)
D^T tiles stay resident in SBUF; damp (exp(-fsq*s[b])) computed once per batch
on the ACT engine and fused into the stage-2 PSUM eviction on the DVE.
"""

import os
import sys

import numpy as np

try:
    import concourse.bass as bass
except ImportError:  # fallback if PYTHONPATH not set in the grading env
    sys.path.insert(0, "/opt/trn_rl_repo")
    import concourse.bass as bass

import concourse.bacc as bacc
import concourse.mybir as mybir
import concourse.tile as tile
from contextlib import ExitStack
from concourse.bass_utils import run_bass_kernel_spmd

N = 512
N_CORES = 8
B = 64
C = 3
B_PER = B // N_CORES          # 8 batches per core
IMGS = B_PER * C              # 24 images per core
NB = N // 128                 # 4 partition blocks per image dim

F32 = mybir.dt.float32
# float32r: fp32 rounded to an 11-bit mantissa (low 12 bits zero), runs the
# PE at 1 cycle/row for moving dim >= 256 (vs 4 cycles/row for plain fp32).
# The BIR verifier requires every matmul-input AP and its producer's output
# AP to be float32r-typed, so the whole input path is declared float32r.
USE_F32R = os.environ.get("DCT_MM_DT", "f32r") == "f32r"
MM_DT = mybir.dt.float32r if USE_F32R else F32

TRACE = False          # test.py flips this to get exec_time_ns
LAST_RESULTS = None    # test.py reads profile info from here

_program = None


def _build_program():
    nc = bacc.Bacc()
    x = nc.declare_dram_parameter("x", [IMGS, N, N], MM_DT, isOutput=False)
    s = nc.declare_dram_parameter("s", [B_PER, 128, 1], F32, isOutput=False)
    dtm = nc.declare_dram_parameter("dtm", [N, N], MM_DT, isOutput=False)  # D^T
    fsqn = nc.declare_dram_parameter("fsqn", [N, N], F32, isOutput=False)  # -fsq
    out = nc.declare_dram_parameter("out", [IMGS, N, N], F32, isOutput=True)

    EXP = mybir.ActivationFunctionType.Exp
    COPY = mybir.ActivationFunctionType.Copy

    with tile.TileContext(nc) as tc, ExitStack() as ctx:
        const = ctx.enter_context(tc.tile_pool(name="const", bufs=1))
        xp = ctx.enter_context(tc.tile_pool(name="xp", bufs=3))
        yp = ctx.enter_context(tc.tile_pool(name="yp", bufs=2))
        zp = ctx.enter_context(tc.tile_pool(name="zp", bufs=3))
        pp = ctx.enter_context(tc.tile_pool(name="pp", bufs=4, space="PSUM"))

        # Constants, each in ONE batched DMA: D^T row-blocks, -fsq row-blocks,
        # and the per-batch damp scales.
        dt_all = const.tile([128, NB, N], MM_DT, name="dt_all", tag="dt_all")
        nc.sync.dma_start(dt_all[:], dtm.rearrange("(hb p) k -> p hb k", hb=NB))
        dt_t = [dt_all[:, hb, :] for hb in range(NB)]

        fq_all = const.tile([128, NB, N], F32, name="fq_all", tag="fq_all")
        nc.sync.dma_start(fq_all[:], fsqn.rearrange("(kb p) w -> p kb w", kb=NB))

        s_all = const.tile([128, B_PER, 1], F32, name="s_all", tag="s_all")
        nc.sync.dma_start(s_all[:], s.rearrange("b p one -> p b one"))

        # damp[b][kb] = exp(-fsq * s[b]), shared across the 3 channels.
        damp = []
        for b in range(B_PER):
            row = []
            for kb in range(NB):
                dmp = const.tile([128, N], F32, name=f"damp{b}_{kb}",
                                 tag=f"damp{b}_{kb}")
                nc.scalar.activation(dmp[:], fq_all[:, kb, :], EXP,
                                     scale=s_all[:, b, :])
                row.append(dmp)
            damp.append(row)

        for img in range(IMGS):
            b = img // C
            # X as [p, hb, w]: partition block hb holds rows hb*128..+128.
            xt = xp.tile([128, NB, N], MM_DT, name="xt", tag="xt")
            nc.sync.dma_start(xt[:], x[img].rearrange("(hb p) w -> p hb w", hb=NB))

            # Stage 1: Y^T[wb] = sum_hb X[hb,wb]^T @ D^T[hb]
            yts = []
            for wb in range(NB):
                py = pp.tile([128, N], F32, name="py", tag="py")
                for hb in range(NB):
                    nc.tensor.matmul(
                        py[:],
                        xt[:, hb, wb * 128:(wb + 1) * 128],
                        dt_t[hb],
                        start=(hb == 0),
                        stop=(hb == NB - 1),
                    )
                yt = yp.tile([128, N], MM_DT, name=f"yt{wb}", tag=f"yt{wb}")
                nc.scalar.activation(yt[:], py[:], COPY)   # PSUM -> SBUF on ACT
                yts.append(yt)

            # Stage 2: Z[kb] = sum_wb Y[kb,wb] @ D^T[wb], fused * damp on DVE.
            zt = zp.tile([128, NB, N], F32, name="zt", tag="zt")
            for kb in range(NB):
                pz = pp.tile([128, N], F32, name="pz", tag="pz")
                for wb in range(NB):
                    nc.tensor.matmul(
                        pz[:],
                        yts[wb][:, kb * 128:(kb + 1) * 128],
                        dt_t[wb],
                        start=(wb == 0),
                        stop=(wb == NB - 1),
                    )
                nc.vector.tensor_mul(zt[:, kb, :], pz[:], damp[b][kb][:])
            nc.sync.dma_start(out[img].rearrange("(kb p) w -> p kb w", kb=NB),
                              zt[:])
    nc.compile()
    return nc


def _get_program():
    global _program
    if _program is None:
        _program = _build_program()
    return _program


def _round_fp32r(a):
    """Round fp32 to the fp32r grid: 11-bit mantissa, low 12 bits zero (RNE)."""
    u = a.view(np.uint32)
    r = (u + np.uint32(0x7FF) + ((u >> np.uint32(12)) & np.uint32(1))) \
        & np.uint32(0xFFFFF000)
    return r.view(np.float32)


def _host_consts():
    n = np.arange(N, dtype=np.float64)
    k = n
    Dm = np.cos(np.pi * (n[None, :] + 0.5) * k[:, None] / N)
    scale = np.where(k == 0, np.sqrt(1.0 / N), np.sqrt(2.0 / N))
    Dm = Dm * scale[:, None]                       # D[k, n]
    dtm = np.ascontiguousarray(Dm.T).astype(np.float32)   # D^T[n, k]
    freqs = np.pi * np.linspace(0.0, N - 1.0, N) / N
    fsq = freqs[:, None] ** 2 + freqs[None, :] ** 2
    fsqn = np.ascontiguousarray(-fsq).astype(np.float32)
    return dtm, fsqn


def kernel(x, t):
    global LAST_RESULTS
    x = np.ascontiguousarray(x, dtype=np.float32)
    t = np.asarray(t, dtype=np.float32)
    assert x.shape == (B, C, N, N) and t.shape == (B,)

    dtm, fsqn = _host_consts()
    if USE_F32R:
        x = _round_fp32r(x)
        dtm = _round_fp32r(dtm)
    # blur schedule: tt = (0.5 * 40**t)**2 / 2 = 0.125 * 40**(2t)
    s = (0.125 * np.power(40.0, 2.0 * t.astype(np.float64))).astype(np.float32)
    s_rep = np.ascontiguousarray(
        np.repeat(s[:, None], 128, axis=1).reshape(B, 128, 1))

    nc = _get_program()
    in_maps = []
    for core in range(N_CORES):
        xs = np.ascontiguousarray(
            x[core * B_PER:(core + 1) * B_PER].reshape(IMGS, N, N))
        ss = np.ascontiguousarray(s_rep[core * B_PER:(core + 1) * B_PER])
        in_maps.append({"x": xs, "s": ss, "dtm": dtm, "fsqn": fsqn})

    res = run_bass_kernel_spmd(nc, in_maps, list(range(N_CORES)), trace=TRACE)
    LAST_RESULTS = res
    outs = [res.results[core]["out"].reshape(B_PER, C, N, N)
            for core in range(N_CORES)]
    return np.concatenate(outs, axis=0).astype(np.float32)


# revision 15
# speedup vs baseline: 1.0978x; 1.0978x over previous
"""DCT blur (nn_DCTBlur) on Trainium2, 8 NeuronCores, data-parallel over batch.

out[b,c] = (D @ x[b,c] @ D^T) * exp(-fsq * s[b]),  s[b] = 0.125 * 40**(2*t[b])

Per core: 8 batches x 3 channels = 24 images of 512x512.
Each image: two matmul stages on the PE array.
  Stage 1:  Y^T = X^T @ D^T  via matmul(out=Y^T, lhsT=X_blk, rhs=D^T_blk)
  Stage 2:  Z   = Y  @ D^T   via matmul(out=Z,  lhsT=Y^T_blk, rhs=D^T_blk)
D^T tiles stay resident in SBUF; damp (exp(-fsq*s[b])) computed once per batch
on the ACT engine and fused into the stage-2 PSUM eviction on the DVE.
"""

import os
import sys

import numpy as np

try:
    import concourse.bass as bass
except ImportError:  # fallback if PYTHONPATH not set in the grading env
    sys.path.insert(0, "/opt/trn_rl_repo")
    import concourse.bass as bass

import concourse.bacc as bacc
import concourse.mybir as mybir
import concourse.tile as tile
from contextlib import ExitStack
from concourse.bass_utils import run_bass_kernel_spmd

N = 512
N_CORES = 8
B = 64
C = 3
B_PER = B // N_CORES          # 8 batches per core
IMGS = B_PER * C              # 24 images per core
NB = N // 128                 # 4 partition blocks per image dim

F32 = mybir.dt.float32
# float32r: fp32 rounded to an 11-bit mantissa (low 12 bits zero), runs the
# PE at 1 cycle/row for moving dim >= 256 (vs 4 cycles/row for plain fp32).
# The BIR verifier requires every matmul-input AP and its producer's output
# AP to be float32r-typed, so the whole input path is declared float32r.
USE_F32R = os.environ.get("DCT_MM_DT", "f32r") == "f32r"
MM_DT = mybir.dt.float32r if USE_F32R else F32

TRACE = False          # test.py flips this to get exec_time_ns
LAST_RESULTS = None    # test.py reads profile info from here

_program = None


def _build_program():
    nc = bacc.Bacc()
    x = nc.declare_dram_parameter("x", [IMGS, N, N], MM_DT, isOutput=False)
    s = nc.declare_dram_parameter("s", [B_PER, 128, 1], F32, isOutput=False)
    dtm = nc.declare_dram_parameter("dtm", [N, N], MM_DT, isOutput=False)  # D^T
    fsqn = nc.declare_dram_parameter("fsqn", [N, N], F32, isOutput=False)  # -fsq
    out = nc.declare_dram_parameter("out", [IMGS, N, N], F32, isOutput=True)
    # Tiny auxiliary output that keeps the PE-warmup matmuls alive through DCE.
    warm = nc.declare_dram_parameter("warm", [128, 8], F32, isOutput=True)

    EXP = mybir.ActivationFunctionType.Exp
    COPY = mybir.ActivationFunctionType.Copy

    with tile.TileContext(nc) as tc, ExitStack() as ctx:
        const = ctx.enter_context(tc.tile_pool(name="const", bufs=1))
        xp = ctx.enter_context(tc.tile_pool(name="xp", bufs=3))
        yp = ctx.enter_context(tc.tile_pool(name="yp", bufs=2))
        zp = ctx.enter_context(tc.tile_pool(name="zp", bufs=3))
        pp = ctx.enter_context(tc.tile_pool(name="pp", bufs=4, space="PSUM"))

        # Image-0 input first so the PE can start ~2.8us in, then the consts.
        xt0 = xp.tile([128, NB, N], MM_DT, name="xt", tag="xt")
        nc.sync.dma_start(xt0[:], x[0].rearrange("(hb p) w -> p hb w", hb=NB))

        dt_all = const.tile([128, NB, N], MM_DT, name="dt_all", tag="dt_all")
        nc.sync.dma_start(dt_all[:], dtm.rearrange("(hb p) k -> p hb k", hb=NB))
        dt_t = [dt_all[:, hb, :] for hb in range(NB)]

        fq_all = const.tile([128, NB, N], F32, name="fq_all", tag="fq_all")
        nc.sync.dma_start(fq_all[:], fsqn.rearrange("(kb p) w -> p kb w", kb=NB))

        s_all = const.tile([128, B_PER, 1], F32, name="s_all", tag="s_all")
        nc.sync.dma_start(s_all[:], s.rearrange("b p one -> p b one"))

        # PE warmup: ~4us of matmul activity on memset data releases the HAM
        # clock throttle before the real matmuls arrive (no DMA dependency).
        # Plain fp32 (4 cyc/row) so three instructions cover the window.
        wsrc = const.tile([128, N], F32, name="wsrc", tag="wsrc")
        nc.gpsimd.memset(wsrc[:], 0.5)
        wsb = const.tile([128, 8], F32, name="wsb", tag="wsb")
        for i in range(3):
            pwarm = pp.tile([128, N], F32, name="py", tag="py")
            nc.tensor.matmul(pwarm[:], wsrc[:, 0:128], wsrc[:],
                             start=True, stop=True)
            if i == 2:
                nc.vector.tensor_copy(wsb[:], pwarm[:, 0:8])
        nc.sync.dma_start(warm[:], wsb[:])

        damp = [[None] * NB for _ in range(B_PER)]

        for img in range(IMGS):
            b = img // C
            if img % C == 0:
                # damp[b][kb] = exp(-fsq * s[b]), shared across the 3 channels.
                # Emitted at the batch's first image so the ACT engine
                # interleaves them with evictions instead of front-loading.
                for kb in range(NB):
                    dmp = const.tile([128, N], F32, name=f"damp{b}_{kb}",
                                     tag=f"damp{b}_{kb}")
                    nc.scalar.activation(dmp[:], fq_all[:, kb, :], EXP,
                                         scale=s_all[:, b, :])
                    damp[b][kb] = dmp

            # X as [p, hb, w]: partition block hb holds rows hb*128..+128.
            if img == 0:
                xt = xt0
            else:
                xt = xp.tile([128, NB, N], MM_DT, name="xt", tag="xt")
                nc.sync.dma_start(xt[:],
                                  x[img].rearrange("(hb p) w -> p hb w", hb=NB))

            # Stage 1: Y^T[wb] = sum_hb X[hb,wb]^T @ D^T[hb]
            yts = []
            for wb in range(NB):
                py = pp.tile([128, N], F32, name="py", tag="py")
                for hb in range(NB):
                    nc.tensor.matmul(
                        py[:],
                        xt[:, hb, wb * 128:(wb + 1) * 128],
                        dt_t[hb],
                        start=(hb == 0),
                        stop=(hb == NB - 1),
                    )
                yt = yp.tile([128, N], MM_DT, name=f"yt{wb}", tag=f"yt{wb}")
                nc.scalar.activation(yt[:], py[:], COPY)   # PSUM -> SBUF on ACT
                yts.append(yt)

            # Stage 2: Z[kb] = sum_wb Y[kb,wb] @ D^T[wb], fused * damp on DVE.
            zt = zp.tile([128, NB, N], F32, name="zt", tag="zt")
            for kb in range(NB):
                pz = pp.tile([128, N], F32, name="pz", tag="pz")
                for wb in range(NB):
                    nc.tensor.matmul(
                        pz[:],
                        yts[wb][:, kb * 128:(kb + 1) * 128],
                        dt_t[wb],
                        start=(wb == 0),
                        stop=(wb == NB - 1),
                    )
                nc.vector.tensor_mul(zt[:, kb, :], pz[:], damp[b][kb][:])
            nc.sync.dma_start(out[img].rearrange("(kb p) w -> p kb w", kb=NB),
                              zt[:])
    nc.compile()
    return nc


def _get_program():
    global _program
    if _program is None:
        _program = _build_program()
    return _program


def _round_fp32r(a):
    """Round fp32 to the fp32r grid: 11-bit mantissa, low 12 bits zero (RNE)."""
    u = a.view(np.uint32)
    r = (u + np.uint32(0x7FF) + ((u >> np.uint32(12)) & np.uint32(1))) \
        & np.uint32(0xFFFFF000)
    return r.view(np.float32)


def _host_consts():
    n = np.arange(N, dtype=np.float64)
    k = n
    Dm = np.cos(np.pi * (n[None, :] + 0.5) * k[:, None] / N)
    scale = np.where(k == 0, np.sqrt(1.0 / N), np.sqrt(2.0 / N))
    Dm = Dm * scale[:, None]                       # D[k, n]
    dtm = np.ascontiguousarray(Dm.T).astype(np.float32)   # D^T[n, k]
    freqs = np.pi * np.linspace(0.0, N - 1.0, N) / N
    fsq = freqs[:, None] ** 2 + freqs[None, :] ** 2
    fsqn = np.ascontiguousarray(-fsq).astype(np.float32)
    return dtm, fsqn


def kernel(x, t):
    global LAST_RESULTS
    x = np.ascontiguousarray(x, dtype=np.float32)
    t = np.asarray(t, dtype=np.float32)
    assert x.shape == (B, C, N, N) and t.shape == (B,)

    dtm, fsqn = _host_consts()
    if USE_F32R:
        x = _round_fp32r(x)
        dtm = _round_fp32r(dtm)
    # blur schedule: tt = (0.5 * 40**t)**2 / 2 = 0.125 * 40**(2t)
    s = (0.125 * np.power(40.0, 2.0 * t.astype(np.float64))).astype(np.float32)
    s_rep = np.ascontiguousarray(
        np.repeat(s[:, None], 128, axis=1).reshape(B, 128, 1))

    nc = _get_program()
    in_maps = []
    for core in range(N_CORES):
        xs = np.ascontiguousarray(
            x[core * B_PER:(core + 1) * B_PER].reshape(IMGS, N, N))
        ss = np.ascontiguousarray(s_rep[core * B_PER:(core + 1) * B_PER])
        in_maps.append({"x": xs, "s": ss, "dtm": dtm, "fsqn": fsqn})

    res = run_bass_kernel_spmd(nc, in_maps, list(range(N_CORES)), trace=TRACE)
    LAST_RESULTS = res
    outs = [res.results[core]["out"].reshape(B_PER, C, N, N)
            for core in range(N_CORES)]
    return np.concatenate(outs, axis=0).astype(np.float32)


# revision 17
# speedup vs baseline: 1.1157x; 1.0163x over previous
"""DCT blur (nn_DCTBlur) on Trainium2, 8 NeuronCores, data-parallel over batch.

out[b,c] = (D @ x[b,c] @ D^T) * exp(-fsq * s[b]),  s[b] = 0.125 * 40**(2*t[b])

Per core: 8 batches x 3 channels = 24 images of 512x512.
Each image: two matmul stages on the PE array.
  Stage 1:  Y^T = X^T @ D^T  via matmul(out=Y^T, lhsT=X_blk, rhs=D^T_blk)
  Stage 2:  Z   = Y  @ D^T   via matmul(out=Z,  lhsT=Y^T_blk, rhs=D^T_blk)
D^T tiles stay resident in SBUF; damp (exp(-fsq*s[b])) computed once per batch
on the ACT engine and fused into the stage-2 PSUM eviction on the DVE.
"""

import os
import sys

import numpy as np

try:
    import concourse.bass as bass
except ImportError:  # fallback if PYTHONPATH not set in the grading env
    sys.path.insert(0, "/opt/trn_rl_repo")
    import concourse.bass as bass

import concourse.bacc as bacc
import concourse.mybir as mybir
import concourse.tile as tile
from contextlib import ExitStack
from concourse.bass_utils import run_bass_kernel_spmd

N = 512
N_CORES = 8
B = 64
C = 3
B_PER = B // N_CORES          # 8 batches per core
IMGS = B_PER * C              # 24 images per core
NB = N // 128                 # 4 partition blocks per image dim

F32 = mybir.dt.float32
# float32r: fp32 rounded to an 11-bit mantissa (low 12 bits zero), runs the
# PE at 1 cycle/row for moving dim >= 256 (vs 4 cycles/row for plain fp32).
# The BIR verifier requires every matmul-input AP and its producer's output
# AP to be float32r-typed, so the whole input path is declared float32r.
USE_F32R = os.environ.get("DCT_MM_DT", "f32r") == "f32r"
MM_DT = mybir.dt.float32r if USE_F32R else F32

TRACE = False          # test.py flips this to get exec_time_ns
LAST_RESULTS = None    # test.py reads profile info from here

_program = None


def _build_program():
    nc = bacc.Bacc()
    x = nc.declare_dram_parameter("x", [IMGS, N, N], MM_DT, isOutput=False)
    s = nc.declare_dram_parameter("s", [B_PER, 128, 1], F32, isOutput=False)
    dtm = nc.declare_dram_parameter("dtm", [N, N], MM_DT, isOutput=False)  # D^T
    fsqn = nc.declare_dram_parameter("fsqn", [N, N], F32, isOutput=False)  # -fsq
    out = nc.declare_dram_parameter("out", [IMGS, N, N], F32, isOutput=True)
    # Tiny auxiliary output that keeps the PE-warmup matmuls alive through DCE.
    warm = nc.declare_dram_parameter("warm", [128, 8], F32, isOutput=True)

    EXP = mybir.ActivationFunctionType.Exp
    COPY = mybir.ActivationFunctionType.Copy

    with tile.TileContext(nc) as tc, ExitStack() as ctx:
        const = ctx.enter_context(tc.tile_pool(name="const", bufs=1))
        xp = ctx.enter_context(tc.tile_pool(name="xp", bufs=3))
        yp = ctx.enter_context(tc.tile_pool(name="yp", bufs=2))
        zp = ctx.enter_context(tc.tile_pool(name="zp", bufs=3))
        pp = ctx.enter_context(tc.tile_pool(name="pp", bufs=4, space="PSUM"))

        # Head latency: interleave image-0's input chunks with the D^T chunks
        # so the first matmul (needs only chunk hb=0 of each) starts ~2us in.
        # Tile's subtile dependency tracking gates each matmul on just the
        # chunk DMA that covers its slice.
        xt0 = xp.tile([128, NB, N], MM_DT, name="xt", tag="xt")
        dt_all = const.tile([128, NB, N], MM_DT, name="dt_all", tag="dt_all")
        x0v = x[0].rearrange("(hb p) w -> p hb w", hb=NB)
        dtv = dtm.rearrange("(hb p) k -> p hb k", hb=NB)
        for hb in range(NB):
            nc.sync.dma_start(xt0[:, hb, :], x0v[:, hb, :])
            nc.sync.dma_start(dt_all[:, hb, :], dtv[:, hb, :])
        dt_t = [dt_all[:, hb, :] for hb in range(NB)]

        xt1 = xp.tile([128, NB, N], MM_DT, name="xt", tag="xt")
        x1v = x[1].rearrange("(hb p) w -> p hb w", hb=NB)
        for hb in range(NB):
            nc.sync.dma_start(xt1[:, hb, :], x1v[:, hb, :])

        fq_all = const.tile([128, NB, N], F32, name="fq_all", tag="fq_all")
        nc.sync.dma_start(fq_all[:], fsqn.rearrange("(kb p) w -> p kb w", kb=NB))

        s_all = const.tile([128, B_PER, 1], F32, name="s_all", tag="s_all")
        nc.sync.dma_start(s_all[:], s.rearrange("b p one -> p b one"))

        # Keep the tiny "warm" output written (no PE warmup — it cost more
        # than it saved).
        wsb = const.tile([128, 8], F32, name="wsb", tag="wsb")
        nc.gpsimd.memset(wsb[:], 0.0)
        nc.sync.dma_start(warm[:], wsb[:])

        damp = [[None] * NB for _ in range(B_PER)]

        for img in range(IMGS):
            b = img // C
            if img % C == 0:
                # damp[b][kb] = exp(-fsq * s[b]), shared across the 3 channels.
                # Emitted at the batch's first image so the ACT engine
                # interleaves them with evictions instead of front-loading.
                for kb in range(NB):
                    dmp = const.tile([128, N], F32, name=f"damp{b}_{kb}",
                                     tag=f"damp{b}_{kb}")
                    nc.scalar.activation(dmp[:], fq_all[:, kb, :], EXP,
                                         scale=s_all[:, b, :])
                    damp[b][kb] = dmp

            # X as [p, hb, w]: partition block hb holds rows hb*128..+128.
            if img == 0:
                xt = xt0
            elif img == 1:
                xt = xt1
            else:
                xt = xp.tile([128, NB, N], MM_DT, name="xt", tag="xt")
                nc.sync.dma_start(xt[:],
                                  x[img].rearrange("(hb p) w -> p hb w", hb=NB))

            # Stage 1: Y^T[wb] = sum_hb X[hb,wb]^T @ D^T[hb]
            yts = []
            for wb in range(NB):
                py = pp.tile([128, N], F32, name="py", tag="py")
                for hb in range(NB):
                    nc.tensor.matmul(
                        py[:],
                        xt[:, hb, wb * 128:(wb + 1) * 128],
                        dt_t[hb],
                        start=(hb == 0),
                        stop=(hb == NB - 1),
                    )
                yt = yp.tile([128, N], MM_DT, name=f"yt{wb}", tag=f"yt{wb}")
                nc.scalar.activation(yt[:], py[:], COPY)   # PSUM -> SBUF on ACT
                yts.append(yt)

            # Stage 2: Z[kb] = sum_wb Y[kb,wb] @ D^T[wb], fused * damp on DVE.
            zt = zp.tile([128, NB, N], F32, name="zt", tag="zt")
            for kb in range(NB):
                pz = pp.tile([128, N], F32, name="pz", tag="pz")
                for wb in range(NB):
                    nc.tensor.matmul(
                        pz[:],
                        yts[wb][:, kb * 128:(kb + 1) * 128],
                        dt_t[wb],
                        start=(wb == 0),
                        stop=(wb == NB - 1),
                    )
                nc.vector.tensor_mul(zt[:, kb, :], pz[:], damp[b][kb][:])
            nc.sync.dma_start(out[img].rearrange("(kb p) w -> p kb w", kb=NB),
                              zt[:])
    nc.compile()
    return nc


def _get_program():
    global _program
    if _program is None:
        _program = _build_program()
    return _program


def _round_fp32r(a):
    """Round fp32 to the fp32r grid: 11-bit mantissa, low 12 bits zero (RNE)."""
    u = a.view(np.uint32)
    r = (u + np.uint32(0x7FF) + ((u >> np.uint32(12)) & np.uint32(1))) \
        & np.uint32(0xFFFFF000)
    return r.view(np.float32)


def _host_consts():
    n = np.arange(N, dtype=np.float64)
    k = n
    Dm = np.cos(np.pi * (n[None, :] + 0.5) * k[:, None] / N)
    scale = np.where(k == 0, np.sqrt(1.0 / N), np.sqrt(2.0 / N))
    Dm = Dm * scale[:, None]                       # D[k, n]
    dtm = np.ascontiguousarray(Dm.T).astype(np.float32)   # D^T[n, k]
    freqs = np.pi * np.linspace(0.0, N - 1.0, N) / N
    fsq = freqs[:, None] ** 2 + freqs[None, :] ** 2
    fsqn = np.ascontiguousarray(-fsq).astype(np.float32)
    return dtm, fsqn


def kernel(x, t):
    global LAST_RESULTS
    x = np.ascontiguousarray(x, dtype=np.float32)
    t = np.asarray(t, dtype=np.float32)
    assert x.shape == (B, C, N, N) and t.shape == (B,)

    dtm, fsqn = _host_consts()
    if USE_F32R:
        x = _round_fp32r(x)
        dtm = _round_fp32r(dtm)
    # blur schedule: tt = (0.5 * 40**t)**2 / 2 = 0.125 * 40**(2t)
    s = (0.125 * np.power(40.0, 2.0 * t.astype(np.float64))).astype(np.float32)
    s_rep = np.ascontiguousarray(
        np.repeat(s[:, None], 128, axis=1).reshape(B, 128, 1))

    nc = _get_program()
    in_maps = []
    for core in range(N_CORES):
        xs = np.ascontiguousarray(
            x[core * B_PER:(core + 1) * B_PER].reshape(IMGS, N, N))
        ss = np.ascontiguousarray(s_rep[core * B_PER:(core + 1) * B_PER])
        in_maps.append({"x": xs, "s": ss, "dtm": dtm, "fsqn": fsqn})

    res = run_bass_kernel_spmd(nc, in_maps, list(range(N_CORES)), trace=TRACE)
    LAST_RESULTS = res
    outs = [res.results[core]["out"].reshape(B_PER, C, N, N)
            for core in range(N_CORES)]
    return np.concatenate(outs, axis=0).astype(np.float32)


# revision 19
# speedup vs baseline: 1.1215x; 1.0051x over previous
"""DCT blur (nn_DCTBlur) on Trainium2, 8 NeuronCores, data-parallel over batch.

out[b,c] = (D @ x[b,c] @ D^T) * exp(-fsq * s[b]),  s[b] = 0.125 * 40**(2*t[b])

Per core: 8 batches x 3 channels = 24 images of 512x512.

Stage 1 exploits the DCT cosine symmetry D[k, N-1-n] = (-1)^k D[k, n]:
the host packs each image as [X_upper; flip(X_lower)], the kernel forms
E = Xu + Xr (even rows of the basis) and O = Xu - Xr (odd rows), and the
contraction runs over 256 rows instead of 512 - half the PE MAC cycles.
Stage 1 output Y^T is kf-parity-packed [even | odd]; stage 2 is a normal
512-contraction against resident D^T tiles and produces Z with rows in
parity-packed order. The damp table rows are host-permuted to match, and
the output DMA un-interleaves the rows on the way to DRAM.

damp (exp(-fsq*s[b])) is computed once per batch on the ACT engine and
fused into the stage-2 PSUM eviction on the DVE.
"""

import os
import sys

import numpy as np

try:
    import concourse.bass as bass
except ImportError:  # fallback if PYTHONPATH not set in the grading env
    sys.path.insert(0, "/opt/trn_rl_repo")
    import concourse.bass as bass

import concourse.bacc as bacc
import concourse.mybir as mybir
import concourse.tile as tile
from contextlib import ExitStack
from concourse.bass_utils import run_bass_kernel_spmd

N = 512
N_CORES = 8
B = 64
C = 3
B_PER = B // N_CORES          # 8 batches per core
IMGS = B_PER * C              # 24 images per core
NB = N // 128                 # 4 partition blocks per image dim

F32 = mybir.dt.float32
# float32r: fp32 rounded to an 11-bit mantissa (low 12 bits zero), runs the
# PE at 1 cycle/row for moving dim >= 256 (vs 4 cycles/row for plain fp32).
# The BIR verifier requires every matmul-input AP and its producer's output
# AP to be float32r-typed, so the whole input path is declared float32r.
USE_F32R = os.environ.get("DCT_MM_DT", "f32r") == "f32r"
MM_DT = mybir.dt.float32r if USE_F32R else F32

TRACE = False          # test.py flips this to get exec_time_ns
LAST_RESULTS = None    # test.py reads profile info from here

_program = None


def _build_program():
    nc = bacc.Bacc()
    # x is host-packed per image: rows 0:256 = X[0:256], rows 256:512 =
    # X[511:255:-1] (flipped lower half).
    x = nc.declare_dram_parameter("x", [IMGS, N, N], MM_DT, isOutput=False)
    s = nc.declare_dram_parameter("s", [B_PER, 128, 1], F32, isOutput=False)
    # D^T natural, for stage 2.
    dtm = nc.declare_dram_parameter("dtm", [N, N], MM_DT, isOutput=False)
    # Stage-1 parity basis: dtmeo[(par*2+hb)*128+p, ke] = D^T[hb*128+p, 2ke+par]
    dtmeo = nc.declare_dram_parameter("dtmeo", [N, 256], MM_DT, isOutput=False)
    # -fsq with ROWS in parity-packed order (evens then odds).
    fsqn = nc.declare_dram_parameter("fsqn", [N, N], F32, isOutput=False)
    out = nc.declare_dram_parameter("out", [IMGS, N, N], F32, isOutput=True)
    warm = nc.declare_dram_parameter("warm", [128, 8], F32, isOutput=True)

    EXP = mybir.ActivationFunctionType.Exp
    COPY = mybir.ActivationFunctionType.Copy

    with tile.TileContext(nc) as tc, ExitStack() as ctx:
        const = ctx.enter_context(tc.tile_pool(name="const", bufs=1))
        xp = ctx.enter_context(tc.tile_pool(name="xp", bufs=3))
        ep = ctx.enter_context(tc.tile_pool(name="ep", bufs=2))
        yp = ctx.enter_context(tc.tile_pool(name="yp", bufs=2))
        zp = ctx.enter_context(tc.tile_pool(name="zp", bufs=3))
        pp = ctx.enter_context(tc.tile_pool(name="pp", bufs=4, space="PSUM"))

        # Head: image-0 chunks interleaved with the stage-1 basis so the
        # first E/O add + matmul can start as soon as possible.
        xt0 = xp.tile([128, NB, N], MM_DT, name="xt", tag="xt")
        x0v = x[0].rearrange("(c p) w -> p c w", c=NB)
        # order: c0, c2 (E/O chunk 0 sources), then c1, c3
        nc.sync.dma_start(xt0[:, 0, :], x0v[:, 0, :])
        nc.sync.dma_start(xt0[:, 2, :], x0v[:, 2, :])

        dte_all = const.tile([128, 2, 2, 256], MM_DT, name="dte", tag="dte")
        nc.sync.dma_start(
            dte_all[:], dtmeo.rearrange("(par hb p) k -> p par hb k", par=2, hb=2))

        nc.sync.dma_start(xt0[:, 1, :], x0v[:, 1, :])
        nc.sync.dma_start(xt0[:, 3, :], x0v[:, 3, :])

        dt_all = const.tile([128, NB, N], MM_DT, name="dt_all", tag="dt_all")
        nc.sync.dma_start(dt_all[:], dtm.rearrange("(hb p) k -> p hb k", hb=NB))
        dt_t = [dt_all[:, hb, :] for hb in range(NB)]

        xt1 = xp.tile([128, NB, N], MM_DT, name="xt", tag="xt")
        nc.sync.dma_start(xt1[:], x[1].rearrange("(c p) w -> p c w", c=NB))

        fq_all = const.tile([128, NB, N], F32, name="fq_all", tag="fq_all")
        nc.sync.dma_start(fq_all[:], fsqn.rearrange("(kb p) w -> p kb w", kb=NB))

        s_all = const.tile([128, B_PER, 1], F32, name="s_all", tag="s_all")
        nc.sync.dma_start(s_all[:], s.rearrange("b p one -> p b one"))

        wsb = const.tile([128, 8], F32, name="wsb", tag="wsb")
        nc.gpsimd.memset(wsb[:], 0.0)
        nc.sync.dma_start(warm[:], wsb[:])

        damp = [[None] * NB for _ in range(B_PER)]

        for img in range(IMGS):
            b = img // C
            if img % C == 0:
                # damp[b][kb] = exp(-fsq_perm * s[b]), shared by 3 channels.
                for kb in range(NB):
                    dmp = const.tile([128, N], F32, name=f"damp{b}_{kb}",
                                     tag=f"damp{b}_{kb}")
                    nc.scalar.activation(dmp[:], fq_all[:, kb, :], EXP,
                                         scale=s_all[:, b, :])
                    damp[b][kb] = dmp

            if img == 0:
                xt = xt0
            elif img == 1:
                xt = xt1
            else:
                xt = xp.tile([128, NB, N], MM_DT, name="xt", tag="xt")
                nc.sync.dma_start(xt[:],
                                  x[img].rearrange("(c p) w -> p c w", c=NB))

            # E = Xu + Xr on GpSimd, O = Xu - Xr on DVE (parallel engines).
            # Element (p, j, w) pairs chunk j with chunk j+2: row h=j*128+p
            # against packed row 256+h = X[511-h].
            e1 = ep.tile([128, 2, N], MM_DT, name="e1", tag="e1")
            o1 = ep.tile([128, 2, N], MM_DT, name="o1", tag="o1")
            nc.gpsimd.tensor_add(e1[:], xt[:, 0:2, :], xt[:, 2:4, :])
            nc.vector.tensor_sub(o1[:], xt[:, 0:2, :], xt[:, 2:4, :])

            # Stage 1 (half contraction): Y^T[wb][:, par*256+ke]
            #   = sum_h2b EO[par][h2b, wb-slice]^T @ dte[par][h2b]
            yts = []
            for wb in range(NB):
                py = pp.tile([128, N], F32, name="py", tag="py")
                for par, eo in ((0, e1), (1, o1)):
                    for h2b in range(2):
                        nc.tensor.matmul(
                            py[:, par * 256:(par + 1) * 256],
                            eo[:, h2b, wb * 128:(wb + 1) * 128],
                            dte_all[:, par, h2b, :],
                            start=(h2b == 0),
                            stop=(h2b == 1),
                        )
                yt = yp.tile([128, N], MM_DT, name=f"yt{wb}", tag=f"yt{wb}")
                nc.scalar.activation(yt[:], py[:], COPY)   # PSUM -> SBUF on ACT
                yts.append(yt)

            # Stage 2: Z[kbP] = sum_wb Y[kbP, wb] @ D^T[wb]; rows of Z come
            # out in parity-packed order, damp rows are pre-permuted to match.
            zt = zp.tile([128, NB, N], F32, name="zt", tag="zt")
            for kb in range(NB):
                pz = pp.tile([128, N], F32, name="pz", tag="pz")
                for wb in range(NB):
                    nc.tensor.matmul(
                        pz[:],
                        yts[wb][:, kb * 128:(kb + 1) * 128],
                        dt_t[wb],
                        start=(wb == 0),
                        stop=(wb == NB - 1),
                    )
                nc.vector.tensor_mul(zt[:, kb, :], pz[:], damp[b][kb][:])
            # Un-interleave parity rows on the way out:
            # out row = 2*(kb*128+p) + par  <-  zt[:, par*2+kb, :]
            nc.sync.dma_start(
                out[img].rearrange("(kb p two) w -> p two kb w", two=2, p=128),
                zt[:].rearrange("p (two kb) w -> p two kb w", two=2))
    nc.compile()
    return nc


def _get_program():
    global _program
    if _program is None:
        _program = _build_program()
    return _program


def _round_fp32r(a):
    """Round fp32 to the fp32r grid: 11-bit mantissa, low 12 bits zero (RNE)."""
    u = a.view(np.uint32)
    r = (u + np.uint32(0x7FF) + ((u >> np.uint32(12)) & np.uint32(1))) \
        & np.uint32(0xFFFFF000)
    return r.view(np.float32)


def _host_consts():
    n = np.arange(N, dtype=np.float64)
    k = n
    Dm = np.cos(np.pi * (n[None, :] + 0.5) * k[:, None] / N)
    scale = np.where(k == 0, np.sqrt(1.0 / N), np.sqrt(2.0 / N))
    Dm = Dm * scale[:, None]                       # D[k, n]
    dtm = np.ascontiguousarray(Dm.T).astype(np.float32)   # D^T[n, k]
    # Stage-1 parity basis.
    dtmeo = np.empty((N, 256), np.float32)
    for par in range(2):
        for hb in range(2):
            r0 = (par * 2 + hb) * 128
            dtmeo[r0:r0 + 128] = dtm[hb * 128:(hb + 1) * 128, par::2]
    freqs = np.pi * np.linspace(0.0, N - 1.0, N) / N
    fsq = freqs[:, None] ** 2 + freqs[None, :] ** 2
    perm = np.concatenate([np.arange(0, N, 2), np.arange(1, N, 2)])
    fsqn = np.ascontiguousarray(-fsq[perm, :]).astype(np.float32)
    return dtm, dtmeo, fsqn


def kernel(x, t):
    global LAST_RESULTS
    x = np.ascontiguousarray(x, dtype=np.float32)
    t = np.asarray(t, dtype=np.float32)
    assert x.shape == (B, C, N, N) and t.shape == (B,)

    dtm, dtmeo, fsqn = _host_consts()
    if USE_F32R:
        x = _round_fp32r(x)
        dtm = _round_fp32r(dtm)
        dtmeo = _round_fp32r(dtmeo)
    # blur schedule: tt = (0.5 * 40**t)**2 / 2 = 0.125 * 40**(2t)
    s = (0.125 * np.power(40.0, 2.0 * t.astype(np.float64))).astype(np.float32)
    s_rep = np.ascontiguousarray(
        np.repeat(s[:, None], 128, axis=1).reshape(B, 128, 1))

    nc = _get_program()
    in_maps = []
    for core in range(N_CORES):
        xs = x[core * B_PER:(core + 1) * B_PER].reshape(IMGS, N, N)
        # pack: [X_upper; flip(X_lower)] per image
        xs = np.concatenate([xs[:, :N // 2], xs[:, :N // 2 - 1:-1]], axis=1)
        ss = np.ascontiguousarray(s_rep[core * B_PER:(core + 1) * B_PER])
        in_maps.append({"x": np.ascontiguousarray(xs), "s": ss, "dtm": dtm,
                        "dtmeo": dtmeo, "fsqn": fsqn})

    res = run_bass_kernel_spmd(nc, in_maps, list(range(N_CORES)), trace=TRACE)
    LAST_RESULTS = res
    outs = [res.results[core]["out"].reshape(B_PER, C, N, N)
            for core in range(N_CORES)]
    return np.concatenate(outs, axis=0).astype(np.float32)


# revision 22
# speedup vs baseline: 1.2178x; 1.0859x over previous
"""DCT blur (nn_DCTBlur) on Trainium2, 8 NeuronCores, data-parallel over batch.

out[b,c] = (D @ x[b,c] @ D^T) * exp(-fsq * s[b]),  s[b] = 0.125 * 40**(2*t[b])

Per core: 8 batches x 3 channels = 24 images of 512x512.

Stage 1 exploits the DCT cosine symmetry D[k, N-1-n] = (-1)^k D[k, n]:
the host packs each image as [X_upper; flip(X_lower)], the kernel forms
E = Xu + Xr (even rows of the basis) and O = Xu - Xr (odd rows), and the
contraction runs over 256 rows instead of 512 - half the PE MAC cycles.
Stage 1 output Y^T is kf-parity-packed [even | odd]; stage 2 is a normal
512-contraction against resident D^T tiles and produces Z with rows in
parity-packed order. The damp table rows are host-permuted to match, and
the output DMA un-interleaves the rows on the way to DRAM.

damp (exp(-fsq*s[b])) is computed once per batch on the ACT engine and
fused into the stage-2 PSUM eviction on the DVE.
"""

import os
import sys

import numpy as np

try:
    import concourse.bass as bass
except ImportError:  # fallback if PYTHONPATH not set in the grading env
    sys.path.insert(0, "/opt/trn_rl_repo")
    import concourse.bass as bass

import concourse.bacc as bacc
import concourse.mybir as mybir
import concourse.tile as tile
from contextlib import ExitStack
from concourse.bass_utils import run_bass_kernel_spmd

N = 512
N_CORES = 8
B = 64
C = 3
B_PER = B // N_CORES          # 8 batches per core
IMGS = B_PER * C              # 24 images per core
NB = N // 128                 # 4 partition blocks per image dim

F32 = mybir.dt.float32
# float32r: fp32 rounded to an 11-bit mantissa (low 12 bits zero), runs the
# PE at 1 cycle/row for moving dim >= 256 (vs 4 cycles/row for plain fp32).
# The BIR verifier requires every matmul-input AP and its producer's output
# AP to be float32r-typed, so the whole input path is declared float32r.
USE_F32R = os.environ.get("DCT_MM_DT", "f32r") == "f32r"
MM_DT = mybir.dt.float32r if USE_F32R else F32

TRACE = False          # test.py flips this to get exec_time_ns
LAST_RESULTS = None    # test.py reads profile info from here

_program = None


def _build_program():
    nc = bacc.Bacc()
    # x is host-packed per image: rows 0:256 = X[0:256], rows 256:512 =
    # X[511:255:-1] (flipped lower half).
    x = nc.declare_dram_parameter("x", [IMGS, N, N], MM_DT, isOutput=False)
    s = nc.declare_dram_parameter("s", [B_PER, 128, 1], F32, isOutput=False)
    # D^T natural, for stage 2.
    dtm = nc.declare_dram_parameter("dtm", [N, N], MM_DT, isOutput=False)
    # Stage-1 parity basis: dtmeo[(par*2+hb)*128+p, ke] = D^T[hb*128+p, 2ke+par]
    dtmeo = nc.declare_dram_parameter("dtmeo", [N, 256], MM_DT, isOutput=False)
    # -fsq with ROWS in parity-packed order (evens then odds).
    fsqn = nc.declare_dram_parameter("fsqn", [N, N], F32, isOutput=False)
    out = nc.declare_dram_parameter("out", [IMGS, N, N], F32, isOutput=True)
    warm = nc.declare_dram_parameter("warm", [128, 8], F32, isOutput=True)

    EXP = mybir.ActivationFunctionType.Exp
    COPY = mybir.ActivationFunctionType.Copy

    with tile.TileContext(nc) as tc, ExitStack() as ctx:
        const = ctx.enter_context(tc.tile_pool(name="const", bufs=1))
        xp = ctx.enter_context(tc.tile_pool(name="xp", bufs=3))
        ep = ctx.enter_context(tc.tile_pool(name="ep", bufs=3))
        yp = ctx.enter_context(tc.tile_pool(name="yp", bufs=2))
        zp = ctx.enter_context(tc.tile_pool(name="zp", bufs=3))
        pp = ctx.enter_context(tc.tile_pool(name="pp", bufs=4, space="PSUM"))

        # Head: stage-1 parity basis first (small), then image-0 chunks in
        # E/O pairing order so the chunked adds can start early.
        dte_all = const.tile([128, 2, 2, 256], MM_DT, name="dte", tag="dte")
        nc.sync.dma_start(
            dte_all[:], dtmeo.rearrange("(par hb p) k -> p par hb k", par=2, hb=2))

        xt0 = xp.tile([128, NB, N], MM_DT, name="xt", tag="xt")
        x0v = x[0].rearrange("(c p) w -> p c w", c=NB)
        # order: c0, c2 (E/O chunk 0 sources), then c1, c3
        nc.sync.dma_start(xt0[:, 0, :], x0v[:, 0, :])
        nc.sync.dma_start(xt0[:, 2, :], x0v[:, 2, :])
        nc.sync.dma_start(xt0[:, 1, :], x0v[:, 1, :])
        nc.sync.dma_start(xt0[:, 3, :], x0v[:, 3, :])

        dt_all = const.tile([128, NB, N], MM_DT, name="dt_all", tag="dt_all")
        nc.sync.dma_start(dt_all[:], dtm.rearrange("(hb p) k -> p hb k", hb=NB))
        dt_t = [dt_all[:, hb, :] for hb in range(NB)]

        xt1 = xp.tile([128, NB, N], MM_DT, name="xt", tag="xt")
        nc.sync.dma_start(xt1[:], x[1].rearrange("(c p) w -> p c w", c=NB))

        fq_all = const.tile([128, NB, N], F32, name="fq_all", tag="fq_all")
        nc.sync.dma_start(fq_all[:], fsqn.rearrange("(kb p) w -> p kb w", kb=NB))

        s_all = const.tile([128, B_PER, 1], F32, name="s_all", tag="s_all")
        nc.sync.dma_start(s_all[:], s.rearrange("b p one -> p b one"))

        wsb = const.tile([128, 8], F32, name="wsb", tag="wsb")
        nc.gpsimd.memset(wsb[:], 0.0)
        nc.sync.dma_start(warm[:], wsb[:])

        damp = [[None] * NB for _ in range(B_PER)]

        for img in range(IMGS):
            b = img // C
            if img % C == 0:
                # damp[b][kb] = exp(-fsq_perm * s[b]), shared by 3 channels.
                for kb in range(NB):
                    dmp = const.tile([128, N], F32, name=f"damp{b}_{kb}",
                                     tag=f"damp{b}_{kb}")
                    nc.scalar.activation(dmp[:], fq_all[:, kb, :], EXP,
                                         scale=s_all[:, b, :])
                    damp[b][kb] = dmp

            if img == 0:
                xt = xt0
            elif img == 1:
                xt = xt1
            else:
                xt = xp.tile([128, NB, N], MM_DT, name="xt", tag="xt")
                nc.sync.dma_start(xt[:],
                                  x[img].rearrange("(c p) w -> p c w", c=NB))

            # E = Xu + Xr, O = Xu - Xr on the DVE. Element (p, j, w) pairs
            # chunk j with chunk j+2: row h=j*128+p against packed row
            # 256+h = X[511-h]. Image 0 is chunked so the first matmul can
            # start after only half its input has landed.
            e1 = ep.tile([128, 2, N], MM_DT, name="e1", tag="e1")
            o1 = ep.tile([128, 2, N], MM_DT, name="o1", tag="o1")
            if img == 0:
                for j in range(2):
                    nc.vector.tensor_add(e1[:, j, :], xt[:, j, :],
                                         xt[:, j + 2, :])
                    nc.vector.tensor_sub(o1[:, j, :], xt[:, j, :],
                                         xt[:, j + 2, :])
            else:
                nc.vector.tensor_add(e1[:], xt[:, 0:2, :], xt[:, 2:4, :])
                nc.vector.tensor_sub(o1[:], xt[:, 0:2, :], xt[:, 2:4, :])

            # Stage 1 (half contraction): Y^T[wb][:, par*256+ke]
            #   = sum_h2b EO[par][h2b, wb-slice]^T @ dte[par][h2b]
            yts = []
            for wb in range(NB):
                py = pp.tile([128, N], F32, name="py", tag="py")
                for par, eo in ((0, e1), (1, o1)):
                    for h2b in range(2):
                        nc.tensor.matmul(
                            py[:, par * 256:(par + 1) * 256],
                            eo[:, h2b, wb * 128:(wb + 1) * 128],
                            dte_all[:, par, h2b, :],
                            start=(h2b == 0),
                            stop=(h2b == 1),
                        )
                yt = yp.tile([128, N], MM_DT, name=f"yt{wb}", tag=f"yt{wb}")
                nc.scalar.activation(yt[:], py[:], COPY)   # PSUM -> SBUF on ACT
                yts.append(yt)

            # Stage 2: Z[kbP] = sum_wb Y[kbP, wb] @ D^T[wb]; rows of Z come
            # out in parity-packed order, damp rows are pre-permuted to match.
            zt = zp.tile([128, NB, N], F32, name="zt", tag="zt")
            for kb in range(NB):
                pz = pp.tile([128, N], F32, name="pz", tag="pz")
                for wb in range(NB):
                    nc.tensor.matmul(
                        pz[:],
                        yts[wb][:, kb * 128:(kb + 1) * 128],
                        dt_t[wb],
                        start=(wb == 0),
                        stop=(wb == NB - 1),
                    )
                nc.vector.tensor_mul(zt[:, kb, :], pz[:], damp[b][kb][:])
            # Un-interleave parity rows on the way out:
            # out row = 2*(kb*128+p) + par  <-  zt[:, par*2+kb, :]
            nc.sync.dma_start(
                out[img].rearrange("(kb p two) w -> p two kb w", two=2, p=128),
                zt[:].rearrange("p (two kb) w -> p two kb w", two=2))
    nc.compile()
    return nc


def _get_program():
    global _program
    if _program is None:
        _program = _build_program()
    return _program


def _round_fp32r(a):
    """Round fp32 to the fp32r grid: 11-bit mantissa, low 12 bits zero (RNE)."""
    u = a.view(np.uint32)
    r = (u + np.uint32(0x7FF) + ((u >> np.uint32(12)) & np.uint32(1))) \
        & np.uint32(0xFFFFF000)
    return r.view(np.float32)


def _host_consts():
    n = np.arange(N, dtype=np.float64)
    k = n
    Dm = np.cos(np.pi * (n[None, :] + 0.5) * k[:, None] / N)
    scale = np.where(k == 0, np.sqrt(1.0 / N), np.sqrt(2.0 / N))
    Dm = Dm * scale[:, None]                       # D[k, n]
    dtm = np.ascontiguousarray(Dm.T).astype(np.float32)   # D^T[n, k]
    # Stage-1 parity basis.
    dtmeo = np.empty((N, 256), np.float32)
    for par in range(2):
        for hb in range(2):
            r0 = (par * 2 + hb) * 128
            dtmeo[r0:r0 + 128] = dtm[hb * 128:(hb + 1) * 128, par::2]
    freqs = np.pi * np.linspace(0.0, N - 1.0, N) / N
    fsq = freqs[:, None] ** 2 + freqs[None, :] ** 2
    perm = np.concatenate([np.arange(0, N, 2), np.arange(1, N, 2)])
    fsqn = np.ascontiguousarray(-fsq[perm, :]).astype(np.float32)
    return dtm, dtmeo, fsqn


def kernel(x, t):
    global LAST_RESULTS
    x = np.ascontiguousarray(x, dtype=np.float32)
    t = np.asarray(t, dtype=np.float32)
    assert x.shape == (B, C, N, N) and t.shape == (B,)

    dtm, dtmeo, fsqn = _host_consts()
    if USE_F32R:
        x = _round_fp32r(x)
        dtm = _round_fp32r(dtm)
        dtmeo = _round_fp32r(dtmeo)
    # blur schedule: tt = (0.5 * 40**t)**2 / 2 = 0.125 * 40**(2t)
    s = (0.125 * np.power(40.0, 2.0 * t.astype(np.float64))).astype(np.float32)
    s_rep = np.ascontiguousarray(
        np.repeat(s[:, None], 128, axis=1).reshape(B, 128, 1))

    nc = _get_program()
    in_maps = []
    for core in range(N_CORES):
        xs = x[core * B_PER:(core + 1) * B_PER].reshape(IMGS, N, N)
        # pack: [X_upper; flip(X_lower)] per image
        xs = np.concatenate([xs[:, :N // 2], xs[:, :N // 2 - 1:-1]], axis=1)
        ss = np.ascontiguousarray(s_rep[core * B_PER:(core + 1) * B_PER])
        in_maps.append({"x": np.ascontiguousarray(xs), "s": ss, "dtm": dtm,
                        "dtmeo": dtmeo, "fsqn": fsqn})

    res = run_bass_kernel_spmd(nc, in_maps, list(range(N_CORES)), trace=TRACE)
    LAST_RESULTS = res
    outs = [res.results[core]["out"].reshape(B_PER, C, N, N)
            for core in range(N_CORES)]
    return np.concatenate(outs, axis=0).astype(np.float32)
